# revision 21
# baseline (speedup 1.0000x reference)
"""Trainium2 Bass kernel for nn_AELossV2 (loss_fn).

Full inputs -> (pull, push) scalars.

Strategy: data-parallel over batch B=8 across 8 NeuronCores. Core k
processes mask[k] ([2048, 2048] u8, the only large tensor) plus its
[2048]-row slices of the small tensors, producing 5 scalar partials:
  [pull_num, pull_den, diag_masked_cnt, mask_cnt_raw, abssum]
The host sums partials over cores and forms the two output scalars.

Per-core math (s = sigmoid(avg_row), thr = 0.6):
  abssum = sum_ij mask[i,j] * |s[j] - s[i]|     (dist_mask is implicit:
           pairs excluded by dist_mask have s equal in EVERY batch row,
           so their |s[j]-s[i]| term is 0 in every core's sum already)
  count  = sum_ij mask[i,j] - sum_i mask[i,i]   (- host-side correction
           for duplicate s-columns, which never occur for random data)
  push   = (thr*count - abssum) / count
  pull   = sum(tag * c) / sum(c),  tag = softplus(x) - x * (g > 0)

Engine split per [128, 2048] tile of the [N, N] plane:
  ACT:  at = Abs(s_brd - s_col)                (|d| via the free affine)
  DVE:  P  = min(mask_u8, at)                  (masking without a cast:
        u8 1 converts to 1.0 > |d|, so min selects |d|; 0 selects 0)
  PE :  column sums of P accumulated in PSUM   (-> abssum)
  DMA:  gpsimd accum-DMA folds all mask tiles into one u8 accumulator
        (values <= 16) -> count costs no compute-engine time.
"""

import sys
from contextlib import ExitStack

import numpy as np

try:
    import concourse.bass  # noqa: F401
except ImportError:  # pragma: no cover
    sys.path.insert(0, "/opt/trn_rl_repo")

B = 8
N = 2048
P = 128
NT = N // P  # 16 tiles per plane
THR = 0.5 + 0.1
N_CORES = 8
N_PARTIALS = 8  # padded partials vector


def build_kernel():
    import concourse.bass as bass
    import concourse.tile as tile
    from concourse import bacc, mybir

    f16 = mybir.dt.float16
    f32 = mybir.dt.float32
    u8 = mybir.dt.uint8
    AF = mybir.ActivationFunctionType
    OP = mybir.AluOpType
    AX = mybir.AxisListType

    nc = bacc.Bacc("TRN2", target_bir_lowering=False, debug=False)

    mask_d = nc.dram_tensor("mask", [N, N], u8, kind="ExternalInput")
    avg_d = nc.dram_tensor("avg", [NT, P], f32, kind="ExternalInput")
    x_d = nc.dram_tensor("x", [NT, P], f32, kind="ExternalInput")
    g_d = nc.dram_tensor("g", [NT, P], f32, kind="ExternalInput")
    c_d = nc.dram_tensor("cen", [NT, P], f32, kind="ExternalInput")
    out_d = nc.dram_tensor("out", [N_PARTIALS, 1], f32, kind="ExternalOutput")

    srow_d = nc.dram_tensor("srow_scratch", [N], f16)
    part_d = nc.dram_tensor("part_scratch", [P, N_PARTIALS], f32)

    with tile.TileContext(nc) as tc, ExitStack() as ctx:
        const = ctx.enter_context(tc.tile_pool(name="const", bufs=1))
        mpool = ctx.enter_context(tc.tile_pool(name="masku8", bufs=1))
        apool = ctx.enter_context(tc.tile_pool(name="absd", bufs=4))
        ppool = ctx.enter_context(tc.tile_pool(name="prod", bufs=4))
        cpool = ctx.enter_context(tc.tile_pool(name="cntscratch", bufs=2))
        pspool = ctx.enter_context(
            tc.tile_pool(name="ps", bufs=1, space=bass.MemorySpace.PSUM)
        )

        # ---------------- setup: s in two layouts ----------------
        # [16, 128] tiles hold vec[t*128 + p] at (t, p): contiguous DMA
        avg_sb = const.tile([NT, P], f32)
        nc.sync.dma_start(avg_sb[:], avg_d.ap())


        # sigmoid(x) = 1 / (1 + exp(-x)) -- keeps the ACT table set to
        # natural_log_exp_and_others (Softplus has no table entry).
        s_exp = const.tile([NT, P], f32)
        nc.scalar.activation(s_exp[:], avg_sb[:], AF.Exp, scale=-1.0)
        s_ep1 = const.tile([NT, P], f32)
        nc.vector.tensor_scalar(
            out=s_ep1[:], in0=s_exp[:], scalar1=1.0, scalar2=None, op0=OP.add
        )
        s_f32 = const.tile([NT, P], f32)
        nc.vector.reciprocal(s_f32[:], s_ep1[:])
        s16row = const.tile([NT, P], f16)
        nc.vector.tensor_copy(s16row[:], s_f32[:])
        # broadcast row layout via DRAM bounce; the big broadcast read goes
        # through the software DGE so the two HWDGE queues stay free for mask
        nc.sync.dma_start(srow_d.ap().rearrange("(t p) -> t p", p=P), s16row[:])
        s_brd = const.tile([P, N], f16)
        nc.gpsimd.dma_start(
            s_brd[:], srow_d.ap().unsqueeze(0).broadcast_to([P, N])
        )
        # -s in [128, 16] per-tile-scalar layout (fp16-rounded values so the
        # diagonal |d| is exactly 0); transpose via the DMA xbar
        s16neg = const.tile([NT, P], f16)
        nc.vector.tensor_scalar(
            out=s16neg[:], in0=s16row[:], scalar1=-1.0, scalar2=None, op0=OP.mult
        )
        sneg16_col = const.tile([P, NT], f16)
        nc.scalar.dma_start_transpose(sneg16_col[:], s16neg[:])
        sneg_col = const.tile([P, NT], f32)
        nc.vector.tensor_copy(sneg_col[:], sneg16_col[:])

        # mask loads split across the two HWDGE queues (sync + scalar),
        # queued behind only the tiny s-chain transfers
        mts = []
        for t in range(NT):
            mt = mpool.tile([P, N], u8, tag=f"mt{t}")
            eng = nc.sync if t % 2 == 0 else nc.scalar
            eng.dma_start(mt[:], mask_d.ap()[t * P : (t + 1) * P, :])
            mts.append(mt)

        partials = const.tile([P, N_PARTIALS], f32)
        nc.vector.memset(partials[:], 0.0)
        ones = const.tile([P, 1], f16)
        nc.vector.memset(ones[:], 1.0)
        NCH = 4  # count-accumulation chains (4 consecutive tiles each)
        acc8s = []
        for c in range(NCH):
            acc8_c = const.tile([P, N], u8, tag=f"acc8_{c}")
            acc8s.append(acc8_c)
        cnt_cols = const.tile([P, NCH], f32)

        # ---------------- pull term (tiny) ----------------
        x_sb = const.tile([NT, P], f32)
        g_sb = const.tile([NT, P], f32)
        c_sb = const.tile([NT, P], f32)
        nc.gpsimd.dma_start(x_sb[:], x_d.ap())
        nc.gpsimd.dma_start(g_sb[:], g_d.ap())
        nc.gpsimd.dma_start(c_sb[:], c_d.ap())
        # softplus(x) = ln(1 + exp(x))
        sp_e = const.tile([NT, P], f32)
        nc.scalar.activation(sp_e[:], x_sb[:], AF.Exp)
        sp_e1 = const.tile([NT, P], f32)
        nc.vector.tensor_scalar(
            out=sp_e1[:], in0=sp_e[:], scalar1=1.0, scalar2=None, op0=OP.add
        )
        sp = const.tile([NT, P], f32)
        nc.scalar.activation(sp[:], sp_e1[:], AF.Ln)
        tgt = const.tile([NT, P], f32)
        nc.vector.tensor_scalar(
            out=tgt[:], in0=g_sb[:], scalar1=0.0, scalar2=None, op0=OP.is_gt
        )
        xt = const.tile([NT, P], f32)
        nc.vector.tensor_tensor(out=xt[:], in0=x_sb[:], in1=tgt[:], op=OP.mult)
        tag = const.tile([NT, P], f32)
        nc.vector.tensor_tensor(out=tag[:], in0=sp[:], in1=xt[:], op=OP.subtract)
        wt = const.tile([NT, P], f32)
        nc.vector.tensor_tensor(out=wt[:], in0=tag[:], in1=c_sb[:], op=OP.mult)
        nc.vector.tensor_reduce(
            out=partials[0:NT, 0:1], in_=wt[:], axis=AX.X, op=OP.add
        )
        nc.vector.tensor_reduce(
            out=partials[0:NT, 1:2], in_=c_sb[:], axis=AX.X, op=OP.add
        )

        # ---------------- the [N, N] plane loop ----------------
        psum_abs = pspool.tile([1, N], f32)
        for t in range(NT):
            mt = mts[t]
            # count accumulation on the DMA engines (software DGE); the
            # first transfer of each chain is a plain copy (no memset)
            nc.gpsimd.dma_start(
                acc8s[t // 4][:], mt[:],
                accum_op=(OP.bypass if t % 4 == 0 else OP.add),
            )
            # |s_j - s_i| on ACT: Abs(1.0 * s_brd + (-s_col))
            at = apool.tile([P, N], f16)
            nc.scalar.activation(
                at[:], s_brd[:], AF.Abs, bias=sneg_col[:, t : t + 1]
            )
            # masked |d| on DVE: mask==1 -> 1.0 > |d|, min picks |d|
            pt_ = ppool.tile([P, N], f16)
            nc.vector.tensor_tensor(out=pt_[:], in0=mt[:], in1=at[:], op=OP.min)
            # column sums into PSUM accumulator
            for c4 in range(4):
                nc.tensor.matmul(
                    psum_abs[0:1, c4 * 512 : (c4 + 1) * 512],
                    ones[:],
                    pt_[:, c4 * 512 : (c4 + 1) * 512],
                    start=(t == 0),
                    stop=(t == NT - 1),
                )
            if t % 4 == 3:
                # fold the finished chain: cast + count row-sums on ACT
                c = t // 4
                csc = cpool.tile([P, N], f16, tag="csc")
                nc.scalar.activation(
                    csc[:], acc8s[c][:], AF.Copy,
                    accum_out=cnt_cols[:, c : c + 1],
                )

        # masked diagonal count -> partials[:, 2]
        diag_u8 = const.tile([P, NT], u8)
        diag_ap = mask_d.ap().rearrange("i j -> (i j)")[:: N + 1].rearrange(
            "(p t) -> p t", t=NT
        )
        nc.sync.dma_start(diag_u8[:], diag_ap)
        diag_f = const.tile([P, NT], f32)
        nc.vector.tensor_copy(diag_f[:], diag_u8[:])
        nc.vector.tensor_reduce(
            out=partials[:, 2:3], in_=diag_f[:], axis=AX.X, op=OP.add
        )



        # ---------------- final reductions ----------------
        nc.vector.tensor_reduce(
            out=partials[0:1, 4:5], in_=psum_abs[:], axis=AX.X, op=OP.add
        )
        nc.vector.tensor_reduce(
            out=partials[:, 3:4], in_=cnt_cols[:], axis=AX.X, op=OP.add
        )

        # transpose partials via DRAM bounce, reduce to [8, 1]
        nc.sync.dma_start(part_d.ap(), partials[:])
        pt8 = const.tile([N_PARTIALS, P], f32)
        nc.sync.dma_start(pt8[:], part_d.ap().rearrange("p c -> c p"))
        out_sb = const.tile([N_PARTIALS, 1], f32)
        nc.vector.tensor_reduce(out=out_sb[:], in_=pt8[:], axis=AX.X, op=OP.add)
        nc.sync.dma_start(out_d.ap(), out_sb[:])

    nc.compile()
    return nc


_NC_CACHE = None


def _get_nc():
    global _NC_CACHE
    if _NC_CACHE is None:
        _NC_CACHE = build_kernel()
    return _NC_CACHE


def _make_in_maps(
    lof_tag_img, lof_tag_avg_img, lof_tag_avg_gather_img, mask, centerness_img
):
    in_maps = []
    for k in range(N_CORES):
        in_maps.append(
            {
                "mask": np.ascontiguousarray(mask[k]).view(np.uint8),
                "avg": np.ascontiguousarray(
                    lof_tag_avg_img[k], dtype=np.float32
                ).reshape(NT, P),
                "x": np.ascontiguousarray(
                    lof_tag_img[k], dtype=np.float32
                ).reshape(NT, P),
                "g": np.ascontiguousarray(
                    lof_tag_avg_gather_img[k], dtype=np.float32
                ).reshape(NT, P),
                "cen": np.ascontiguousarray(
                    centerness_img[k], dtype=np.float32
                ).reshape(NT, P),
            }
        )
    return in_maps


def _dup_column_correction(avg, mask):
    """count correction for duplicate sigmoid columns (all-batch-equal
    pairs beyond the diagonal). Zero for generic random inputs."""
    s = (1.0 / (1.0 + np.exp(-avg.astype(np.float32)))).astype(np.float32)
    cols = np.ascontiguousarray(s.T)  # [N, B]
    _, inv, counts = np.unique(
        cols.view([("", cols.dtype)] * cols.shape[1]).ravel(),
        return_inverse=True,
        return_counts=True,
    )
    corr = 0.0
    if np.any(counts > 1):
        for gid in np.nonzero(counts > 1)[0]:
            idx = np.nonzero(inv == gid)[0]
            for i in idx:
                for j in idx:
                    if i != j:
                        corr += float(mask[:, i, j].sum())
    return corr


def _combine(partials_per_core, avg, mask):
    tot = np.sum(
        [p.reshape(-1).astype(np.float64) for p in partials_per_core], axis=0
    )
    pull_num, pull_den, diag_cnt, cnt_raw, abssum = tot[:5]
    pull = pull_num / pull_den
    count = cnt_raw - diag_cnt - _dup_column_correction(avg, mask)
    if count > 0:
        push = (THR * count - abssum) / count
    else:
        push = 0.0
    return np.float32(pull), np.float32(push)


def kernel(lof_tag_img, lof_tag_avg_img, lof_tag_avg_gather_img, mask, centerness_img):
    from concourse import bass_utils

    nc = _get_nc()
    in_maps = _make_in_maps(
        lof_tag_img, lof_tag_avg_img, lof_tag_avg_gather_img, mask, centerness_img
    )
    res = bass_utils.run_bass_kernel_spmd(
        nc, in_maps, core_ids=list(range(N_CORES))
    )
    partials = [res.results[k]["out"] for k in range(N_CORES)]
    return _combine(
        partials, np.asarray(lof_tag_avg_img), np.asarray(mask)
    )


# revision 22
# speedup vs baseline: 1.0051x; 1.0051x over previous
"""Trainium2 Bass kernel for nn_AELossV2 (loss_fn).

Full inputs -> (pull, push) scalars.

Strategy: data-parallel over batch B=8 across 8 NeuronCores. Core k
processes mask[k] ([2048, 2048] u8, the only large tensor) plus its
[2048]-row slices of the small tensors, producing 5 scalar partials:
  [pull_num, pull_den, diag_masked_cnt, mask_cnt_raw, abssum]
The host sums partials over cores and forms the two output scalars.

Per-core math (s = sigmoid(avg_row), thr = 0.6):
  abssum = sum_ij mask[i,j] * |s[j] - s[i]|     (dist_mask is implicit:
           pairs excluded by dist_mask have s equal in EVERY batch row,
           so their |s[j]-s[i]| term is 0 in every core's sum already)
  count  = sum_ij mask[i,j] - sum_i mask[i,i]   (- host-side correction
           for duplicate s-columns, which never occur for random data)
  push   = (thr*count - abssum) / count
  pull   = sum(tag * c) / sum(c),  tag = softplus(x) - x * (g > 0)

Engine split per [128, 2048] tile of the [N, N] plane:
  ACT:  at = Abs(s_brd - s_col)                (|d| via the free affine)
  DVE:  P  = min(mask_u8, at)                  (masking without a cast:
        u8 1 converts to 1.0 > |d|, so min selects |d|; 0 selects 0)
  PE :  column sums of P accumulated in PSUM   (-> abssum)
  DMA:  gpsimd accum-DMA folds all mask tiles into one u8 accumulator
        (values <= 16) -> count costs no compute-engine time.
"""

import sys
from contextlib import ExitStack

import numpy as np

try:
    import concourse.bass  # noqa: F401
except ImportError:  # pragma: no cover
    sys.path.insert(0, "/opt/trn_rl_repo")

B = 8
N = 2048
P = 128
NT = N // P  # 16 tiles per plane
THR = 0.5 + 0.1
N_CORES = 8
N_PARTIALS = 8  # padded partials vector


def build_kernel():
    import concourse.bass as bass
    import concourse.tile as tile
    from concourse import bacc, mybir

    f16 = mybir.dt.float16
    f32 = mybir.dt.float32
    u8 = mybir.dt.uint8
    AF = mybir.ActivationFunctionType
    OP = mybir.AluOpType
    AX = mybir.AxisListType

    nc = bacc.Bacc("TRN2", target_bir_lowering=False, debug=False)

    mask_d = nc.dram_tensor("mask", [N, N], u8, kind="ExternalInput")
    avg_d = nc.dram_tensor("avg", [NT, P], f32, kind="ExternalInput")
    x_d = nc.dram_tensor("x", [NT, P], f32, kind="ExternalInput")
    g_d = nc.dram_tensor("g", [NT, P], f32, kind="ExternalInput")
    c_d = nc.dram_tensor("cen", [NT, P], f32, kind="ExternalInput")
    out_d = nc.dram_tensor("out", [N_PARTIALS, 1], f32, kind="ExternalOutput")

    srow_d = nc.dram_tensor("srow_scratch", [N], f16)
    part_d = nc.dram_tensor("part_scratch", [P, N_PARTIALS], f32)

    with tile.TileContext(nc) as tc, ExitStack() as ctx:
        const = ctx.enter_context(tc.tile_pool(name="const", bufs=1))
        mpool = ctx.enter_context(tc.tile_pool(name="masku8", bufs=1))
        apool = ctx.enter_context(tc.tile_pool(name="absd", bufs=4))
        ppool = ctx.enter_context(tc.tile_pool(name="prod", bufs=4))
        cpool = ctx.enter_context(tc.tile_pool(name="cntscratch", bufs=2))
        pspool = ctx.enter_context(
            tc.tile_pool(name="ps", bufs=1, space=bass.MemorySpace.PSUM)
        )

        # ---------------- setup: s in two layouts ----------------
        # [16, 128] tiles hold vec[t*128 + p] at (t, p): contiguous DMA
        avg_sb = const.tile([NT, P], f32)
        nc.sync.dma_start(avg_sb[:], avg_d.ap())


        # sigmoid(x) = 1 / (1 + exp(-x)) -- keeps the ACT table set to
        # natural_log_exp_and_others (Softplus has no table entry).
        s_exp = const.tile([NT, P], f32)
        nc.scalar.activation(s_exp[:], avg_sb[:], AF.Exp, scale=-1.0)
        s_ep1 = const.tile([NT, P], f32)
        nc.vector.tensor_scalar(
            out=s_ep1[:], in0=s_exp[:], scalar1=1.0, scalar2=None, op0=OP.add
        )
        s_f32 = const.tile([NT, P], f32)
        nc.vector.reciprocal(s_f32[:], s_ep1[:])
        s16row = const.tile([NT, P], f16)
        nc.vector.tensor_copy(s16row[:], s_f32[:])
        # broadcast row layout: bounce through DRAM to get s as a single
        # [1, 2048] partition row, then rank-1 broadcast on the PE
        # (ones[128] x s_row) -> PSUM -> SBUF. No slow stride-0 DMA.
        nc.sync.dma_start(srow_d.ap().rearrange("(t p) -> t p", p=P), s16row[:])
        srow_sb = const.tile([1, N], f16)
        nc.sync.dma_start(srow_sb[:], srow_d.ap().unsqueeze(0))
        ones_bc = const.tile([1, P], f16)
        nc.vector.memset(ones_bc[:], 1.0)
        s_brd = const.tile([P, N], f16)
        with tc.tile_pool(name="psbrd", bufs=1, space=bass.MemorySpace.PSUM) as psb:
            psum_brd = psb.tile([P, N], f32)
            for c4 in range(4):
                nc.tensor.matmul(
                    psum_brd[:, c4 * 512 : (c4 + 1) * 512],
                    ones_bc[:],
                    srow_sb[0:1, c4 * 512 : (c4 + 1) * 512],
                    start=True,
                    stop=True,
                )
            nc.vector.tensor_copy(s_brd[:], psum_brd[:])
        # -s in [128, 16] per-tile-scalar layout (fp16-rounded values so the
        # diagonal |d| is exactly 0); transpose via the DMA xbar
        s16neg = const.tile([NT, P], f16)
        nc.vector.tensor_scalar(
            out=s16neg[:], in0=s16row[:], scalar1=-1.0, scalar2=None, op0=OP.mult
        )
        sneg16_col = const.tile([P, NT], f16)
        nc.scalar.dma_start_transpose(sneg16_col[:], s16neg[:])
        sneg_col = const.tile([P, NT], f32)
        nc.vector.tensor_copy(sneg_col[:], sneg16_col[:])

        # mask loads split across the two HWDGE queues (sync + scalar),
        # queued behind only the tiny s-chain transfers
        mts = []
        for t in range(NT):
            mt = mpool.tile([P, N], u8, tag=f"mt{t}")
            eng = nc.sync if t % 2 == 0 else nc.scalar
            eng.dma_start(mt[:], mask_d.ap()[t * P : (t + 1) * P, :])
            mts.append(mt)

        partials = const.tile([P, N_PARTIALS], f32)
        nc.vector.memset(partials[:], 0.0)
        ones = const.tile([P, 1], f16)
        nc.vector.memset(ones[:], 1.0)
        NCH = 4  # count-accumulation chains (4 consecutive tiles each)
        acc8s = []
        for c in range(NCH):
            acc8_c = const.tile([P, N], u8, tag=f"acc8_{c}")
            acc8s.append(acc8_c)
        cnt_cols = const.tile([P, NCH], f32)

        # ---------------- pull term (tiny) ----------------
        x_sb = const.tile([NT, P], f32)
        g_sb = const.tile([NT, P], f32)
        c_sb = const.tile([NT, P], f32)
        nc.gpsimd.dma_start(x_sb[:], x_d.ap())
        nc.gpsimd.dma_start(g_sb[:], g_d.ap())
        nc.gpsimd.dma_start(c_sb[:], c_d.ap())
        # softplus(x) = ln(1 + exp(x))
        sp_e = const.tile([NT, P], f32)
        nc.scalar.activation(sp_e[:], x_sb[:], AF.Exp)
        sp_e1 = const.tile([NT, P], f32)
        nc.vector.tensor_scalar(
            out=sp_e1[:], in0=sp_e[:], scalar1=1.0, scalar2=None, op0=OP.add
        )
        sp = const.tile([NT, P], f32)
        nc.scalar.activation(sp[:], sp_e1[:], AF.Ln)
        tgt = const.tile([NT, P], f32)
        nc.vector.tensor_scalar(
            out=tgt[:], in0=g_sb[:], scalar1=0.0, scalar2=None, op0=OP.is_gt
        )
        xt = const.tile([NT, P], f32)
        nc.vector.tensor_tensor(out=xt[:], in0=x_sb[:], in1=tgt[:], op=OP.mult)
        tag = const.tile([NT, P], f32)
        nc.vector.tensor_tensor(out=tag[:], in0=sp[:], in1=xt[:], op=OP.subtract)
        wt = const.tile([NT, P], f32)
        nc.vector.tensor_tensor(out=wt[:], in0=tag[:], in1=c_sb[:], op=OP.mult)
        nc.vector.tensor_reduce(
            out=partials[0:NT, 0:1], in_=wt[:], axis=AX.X, op=OP.add
        )
        nc.vector.tensor_reduce(
            out=partials[0:NT, 1:2], in_=c_sb[:], axis=AX.X, op=OP.add
        )

        # ---------------- the [N, N] plane loop ----------------
        psum_abs = pspool.tile([1, N], f32)
        for t in range(NT):
            mt = mts[t]
            # count accumulation on the DMA engines (software DGE); the
            # first transfer of each chain is a plain copy (no memset)
            nc.gpsimd.dma_start(
                acc8s[t // 4][:], mt[:],
                accum_op=(OP.bypass if t % 4 == 0 else OP.add),
            )
            # |s_j - s_i| on ACT: Abs(1.0 * s_brd + (-s_col))
            at = apool.tile([P, N], f16)
            nc.scalar.activation(
                at[:], s_brd[:], AF.Abs, bias=sneg_col[:, t : t + 1]
            )
            # masked |d| on DVE: mask==1 -> 1.0 > |d|, min picks |d|
            pt_ = ppool.tile([P, N], f16)
            nc.vector.tensor_tensor(out=pt_[:], in0=mt[:], in1=at[:], op=OP.min)
            # column sums into PSUM accumulator
            for c4 in range(4):
                nc.tensor.matmul(
                    psum_abs[0:1, c4 * 512 : (c4 + 1) * 512],
                    ones[:],
                    pt_[:, c4 * 512 : (c4 + 1) * 512],
                    start=(t == 0),
                    stop=(t == NT - 1),
                )
            if t % 4 == 3:
                # fold the finished chain: cast + count row-sums on ACT
                c = t // 4
                csc = cpool.tile([P, N], f16, tag="csc")
                nc.scalar.activation(
                    csc[:], acc8s[c][:], AF.Copy,
                    accum_out=cnt_cols[:, c : c + 1],
                )

        # masked diagonal count -> partials[:, 2]
        diag_u8 = const.tile([P, NT], u8)
        diag_ap = mask_d.ap().rearrange("i j -> (i j)")[:: N + 1].rearrange(
            "(p t) -> p t", t=NT
        )
        nc.sync.dma_start(diag_u8[:], diag_ap)
        diag_f = const.tile([P, NT], f32)
        nc.vector.tensor_copy(diag_f[:], diag_u8[:])
        nc.vector.tensor_reduce(
            out=partials[:, 2:3], in_=diag_f[:], axis=AX.X, op=OP.add
        )



        # ---------------- final reductions ----------------
        nc.vector.tensor_reduce(
            out=partials[0:1, 4:5], in_=psum_abs[:], axis=AX.X, op=OP.add
        )
        nc.vector.tensor_reduce(
            out=partials[:, 3:4], in_=cnt_cols[:], axis=AX.X, op=OP.add
        )

        # transpose partials via DRAM bounce, reduce to [8, 1]
        nc.sync.dma_start(part_d.ap(), partials[:])
        pt8 = const.tile([N_PARTIALS, P], f32)
        nc.sync.dma_start(pt8[:], part_d.ap().rearrange("p c -> c p"))
        out_sb = const.tile([N_PARTIALS, 1], f32)
        nc.vector.tensor_reduce(out=out_sb[:], in_=pt8[:], axis=AX.X, op=OP.add)
        nc.sync.dma_start(out_d.ap(), out_sb[:])

    nc.compile()
    return nc


_NC_CACHE = None


def _get_nc():
    global _NC_CACHE
    if _NC_CACHE is None:
        _NC_CACHE = build_kernel()
    return _NC_CACHE


def _make_in_maps(
    lof_tag_img, lof_tag_avg_img, lof_tag_avg_gather_img, mask, centerness_img
):
    in_maps = []
    for k in range(N_CORES):
        in_maps.append(
            {
                "mask": np.ascontiguousarray(mask[k]).view(np.uint8),
                "avg": np.ascontiguousarray(
                    lof_tag_avg_img[k], dtype=np.float32
                ).reshape(NT, P),
                "x": np.ascontiguousarray(
                    lof_tag_img[k], dtype=np.float32
                ).reshape(NT, P),
                "g": np.ascontiguousarray(
                    lof_tag_avg_gather_img[k], dtype=np.float32
                ).reshape(NT, P),
                "cen": np.ascontiguousarray(
                    centerness_img[k], dtype=np.float32
                ).reshape(NT, P),
            }
        )
    return in_maps


def _dup_column_correction(avg, mask):
    """count correction for duplicate sigmoid columns (all-batch-equal
    pairs beyond the diagonal). Zero for generic random inputs."""
    s = (1.0 / (1.0 + np.exp(-avg.astype(np.float32)))).astype(np.float32)
    cols = np.ascontiguousarray(s.T)  # [N, B]
    _, inv, counts = np.unique(
        cols.view([("", cols.dtype)] * cols.shape[1]).ravel(),
        return_inverse=True,
        return_counts=True,
    )
    corr = 0.0
    if np.any(counts > 1):
        for gid in np.nonzero(counts > 1)[0]:
            idx = np.nonzero(inv == gid)[0]
            for i in idx:
                for j in idx:
                    if i != j:
                        corr += float(mask[:, i, j].sum())
    return corr


def _combine(partials_per_core, avg, mask):
    tot = np.sum(
        [p.reshape(-1).astype(np.float64) for p in partials_per_core], axis=0
    )
    pull_num, pull_den, diag_cnt, cnt_raw, abssum = tot[:5]
    pull = pull_num / pull_den
    count = cnt_raw - diag_cnt - _dup_column_correction(avg, mask)
    if count > 0:
        push = (THR * count - abssum) / count
    else:
        push = 0.0
    return np.float32(pull), np.float32(push)


def kernel(lof_tag_img, lof_tag_avg_img, lof_tag_avg_gather_img, mask, centerness_img):
    from concourse import bass_utils

    nc = _get_nc()
    in_maps = _make_in_maps(
        lof_tag_img, lof_tag_avg_img, lof_tag_avg_gather_img, mask, centerness_img
    )
    res = bass_utils.run_bass_kernel_spmd(
        nc, in_maps, core_ids=list(range(N_CORES))
    )
    partials = [res.results[k]["out"] for k in range(N_CORES)]
    return _combine(
        partials, np.asarray(lof_tag_avg_img), np.asarray(mask)
    )
